# revision 37
# baseline (speedup 1.0000x reference)
"""Trainium2 Bass kernel for GQA attention (B=2,S=2048,D=2048,H=16,KV=4,HD=128)
with RoPE + causal mask, sharded over 8 NeuronCores:
  2-way data parallel over batch x 4-way tensor parallel over KV groups.

Core c = (b, g): b = c // 4, g = c % 4.
Each core computes, for its batch b and KV group g (q heads 4g..4g+3):
  QT_h [HD,S], KT [HD,S] (RoPE'd), V [S,HD]    via matmul vs xT [D,S]
  scoresT [sk,sq] blocks, exp on ScalarE (scale folded), row-sums via an
  all-ones matmul (which also replicates the sums across partitions),
  AV with V tiles stationary -> outT [HD,sq], per-head normalization via
  reciprocal, partial y = attn_norm @ wo_rows[g]; host sums the 4 partials.

matmul(out, lhsT, rhs) = lhsT.T @ rhs, contraction over the partition dim.
All contractions are K=128.  Causality at block granularity: fully-masked
(sk,sq) blocks skipped; diagonal blocks add the mask slice (pattern repeats
every 4 sk-tiles, so only a [512,512] mask transpose is shipped).

Matmul operands are bf16 by default: PE rate is 1 cyc/row for bf16 and
fp32r alike, but bf16 halves the input DMA (phase 1 is otherwise at the
358 GB/s DMA roofline) and doubles DVE elementwise throughput.  All
accumulation stays fp32 in PSUM; softmax reciprocal in fp32.
Softmax denominators: off-diagonal prob tiles are pre-accumulated in two
chains on the otherwise-idle DVE/GpSimd engines so the PE does one
ones-matmul per chain instead of one per tile.
"""

import os
from contextlib import ExitStack

import numpy as np

import concourse.bacc as bacc
import concourse.mybir as mybir
import concourse.tile as tile

# ---------------- problem constants (hardcoded per contract) ----------------
B, S, D = 2, 2048, 2048
H, KV, HD = 16, 4, 128
REP = H // KV            # 4 q heads per kv head
NG = KV                  # 4 tensor-parallel groups
NCORES = 8
THETA = 10000.0
SCALE = 1.0 / float(np.sqrt(HD))

P = 128                  # partition dim
SC = 512                 # moving free-dim chunk (fp32 max / one psum bank)
NDT = S // P             # 16 tiles of 128 along S or D
NCH = S // SC            # 4 chunks of 512 along S
NH = REP                 # 4 q-heads per core

FP32 = mybir.dt.float32
F32R = mybir.dt.float32r
BF16 = mybir.dt.bfloat16

# matmul operand dtype: "bf16" (default; ~1e-3 measured kernel rel err),
# "f32r" (~2.2e-4) or "fp32" (exact but 4 cyc/row on the PE).
MM_MODE = os.environ.get("KERNEL_MM_MODE", "bf16")

_CACHE = {}


def _mdt(mm_mode):
    return {"bf16": BF16, "f32r": F32R, "fp32": FP32}[mm_mode]


def _build_program(mm_mode=MM_MODE, repeat=1):
    MDT = _mdt(mm_mode)

    nc = bacc.Bacc("TRN2", target_bir_lowering=False, debug=False)

    # x is supplied chunk-contiguous: block sc (rows [sc*D:(sc+1)*D]) holds
    # xT[:, sc*SC:(sc+1)*SC], so every DMA row is contiguous in DRAM and
    # the transfer runs at full rate (strided 1KB reads run at half rate).
    xT_d = nc.dram_tensor("xTc", [NCH * D, SC], MDT, kind="ExternalInput").ap()
    wq_d = nc.dram_tensor("wqg", [D, NH * HD], MDT, kind="ExternalInput").ap()
    wk_d = nc.dram_tensor("wkg", [D, HD], MDT, kind="ExternalInput").ap()
    wv_d = nc.dram_tensor("wvg", [D, HD], MDT, kind="ExternalInput").ap()
    wo_d = nc.dram_tensor("wog", [NH * HD, D], MDT, kind="ExternalInput").ap()
    cosT_d = nc.dram_tensor("cosT", [HD, S], MDT, kind="ExternalInput").ap()
    sinrT_d = nc.dram_tensor("sinrotT", [HD, S], MDT, kind="ExternalInput").ap()
    mdiag_d = nc.dram_tensor("maskdiag", [NCH * P, SC], MDT, kind="ExternalInput").ap()
    ident_d = nc.dram_tensor("ident", [P, P], MDT, kind="ExternalInput").ap()
    ones_d = nc.dram_tensor("ones", [P, P], MDT, kind="ExternalInput").ap()
    # y stored bf16, chunk-contiguous by output-column block dci (block
    # dci holds y[:, dci*SC:(dci+1)*SC] as rows [dci*S:(dci+1)*S]) so DMA
    # rows are contiguous and run at full rate; host reassembles + upcasts.
    y_d = nc.dram_tensor("ybc", [NCH * S, SC], MDT, kind="ExternalOutput").ap()

    with tile.TileContext(nc) as tc, ExitStack() as ctx:
        qkv = ctx.enter_context(tc.tile_pool(name="qkv", bufs=1))
        small = ctx.enter_context(tc.tile_pool(name="small", bufs=1))
        # phase-2 pools are persistent (hoisted out of the rep loop) so the
        # final chunk's output projection can spill into the NEXT rep's
        # phase 1, where the PE otherwise idles waiting on x DMA.
        p2 = ctx.enter_context(tc.tile_pool(name="p2", bufs=1))
        ptp = ctx.enter_context(tc.tile_pool(name="pt", bufs=36))
        nrm = ctx.enter_context(tc.tile_pool(name="nrm", bufs=4))
        chp = ctx.enter_context(tc.tile_pool(name="chn", bufs=2))
        psy = ctx.enter_context(tc.tile_pool(name="psy", bufs=2, space="PSUM"))

        # resident Q^T per head, K^T, V tiles
        qt = [qkv.tile([P, S], MDT, tag=f"qt{h}", name=f"qt{h}") for h in range(NH)]
        kt = qkv.tile([P, S], MDT, tag="kt", name="kt")
        v_tiles = [qkv.tile([P, HD], MDT, tag=f"v{k}", name=f"v{k}")
                   for k in range(NDT)]

        ones_sb = small.tile([P, P], MDT, tag="ones")
        ident_sb = small.tile([P, P], MDT, tag="ident")
        mdiag_slab = small.tile([P, NCH * SC], MDT, tag="mds")
        mdiag_sb = [mdiag_slab[:, r * SC:(r + 1) * SC] for r in range(NCH)]
        # RoPE tables are host-derived constants (like ident/ones/maskdiag):
        # resident across reps, loaded once.
        cosT_sb = small.tile([HD, S], MDT, tag="cosT")
        sinrT_sb = small.tile([HD, S], MDT, tag="sinrT")

        wos = p2.tile([P, NH * D], MDT, tag="wos")
        wo_sb = [wos[:, h * D:(h + 1) * D] for h in range(NH)]

        def load_consts():
            nc.gpsimd.dma_start(ones_sb[:], ones_d[:])
            nc.gpsimd.dma_start(cosT_sb[:], cosT_d[:])
            nc.gpsimd.dma_start(sinrT_sb[:], sinrT_d[:])
            nc.sync.dma_start(ident_sb[:], ident_d[:])
            nc.gpsimd.dma_start(
                mdiag_slab[:].rearrange("p (r s) -> p r s", r=NCH),
                mdiag_d.rearrange("(r p) s -> p r s", p=P))

        # ---- pipelined output projection --------------------------------
        # pending = {"c": chunk, "ot": {h: outT tile}, "idx": items done}.
        # Chunk c's oproj matmuls are PE filler, interleaved wherever the
        # PE would otherwise stall: chunks 0-2 into the next chunk's
        # exp-bound attention, chunk 3 into the next rep's DMA-bound
        # phase 1 (the very last one drains after the rep loop).
        oproj_items = [(t, dci) for t in range(SC // P)
                       for dci in range(NCH)]
        yslabs = {}
        pend = {"cur": None}

        def emit_pending(n, copy_engs=None):
            pending = pend["cur"]
            if pending is None:
                return
            items = oproj_items[pending["idx"]:pending["idx"] + n]
            if not items:
                return
            pending["idx"] += len(items)
            q0 = pending["c"] * SC
            otiles = pending["ot"]
            for i, (t, dci) in enumerate(items):
                d0 = dci * SC
                y_ps = psy.tile([P, SC], FP32, tag="y")
                for h in range(NH):
                    nc.tensor.matmul(
                        y_ps[:],
                        otiles[h][:, t * P:(t + 1) * P],
                        wo_sb[h][:, d0:d0 + SC],
                        start=(h == 0), stop=(h == NH - 1),
                    )
                # bf16 staging copy into an independent tile per (t, dci)
                # (a shared slab would serialize the copies: dependency
                # tracking is tile-granular), alternating DVE / ScalarE;
                # stores are contiguous in the chunked y layout and split
                # across the sync / gpsimd descriptor queues
                yb = chp.tile([P, SC], MDT, tag="yb", bufs=4)
                if i % 2 == 0:
                    nc.vector.tensor_copy(yb[:], y_ps[:])
                else:
                    nc.scalar.copy(yb[:], y_ps[:])
                row0 = dci * S + q0 + t * P
                qeng = nc.sync if i % 2 == 0 else nc.gpsimd
                qeng.dma_start(y_d[row0:row0 + P, :], yb[:])

        for rep in range(repeat):
            # ============== phase 1: QKV projection + RoPE ==============
            with tc.tile_pool(name="p1", bufs=1) as p1, \
                 tc.tile_pool(name="xin", bufs=6) as xin, \
                 tc.tile_pool(name="rtmp", bufs=3) as rtmp, \
                 tc.tile_pool(name="ps1", bufs=2, space="PSUM") as ps1:

                # weight slab: tile k of wq lives at slab columns
                # [k*512, (k+1)*512), head slice m at [k*512 + m*128, ...).
                # Loads are split into quarters and spread over two DMA
                # queues (sync + scalar) so the first matmul chain is gated
                # by a fraction, not all, of the phase-1 inputs.
                XQ = NDT // 4   # 4 d-tiles per quarter slab
                wqs = p1.tile([P, NDT * NH * HD], MDT, tag="wqs")
                wks = p1.tile([P, NDT * HD], MDT, tag="wks")
                wvs = p1.tile([P, NDT * HD], MDT, tag="wvs")
                for qq in range(4):
                    r0, r1 = qq * XQ * P, (qq + 1) * XQ * P
                    nc.scalar.dma_start(
                        wqs[:, qq * XQ * NH * HD:(qq + 1) * XQ * NH * HD]
                        .rearrange("p (n m) -> p n m", n=XQ),
                        wq_d[r0:r1, :].rearrange("(n p) m -> p n m", p=P))
                vT = p1.tile([HD, S], MDT, tag="vT")

                for sc in range(NCH):
                    s0 = sc * SC
                    xq_slabs = []
                    for qq in range(4):
                        xs = xin.tile([P, XQ * SC], MDT, tag="x")
                        r0 = sc * D + qq * XQ * P
                        nc.sync.dma_start(
                            xs[:].rearrange("p (n s) -> p n s", n=XQ),
                            xT_d[r0:r0 + XQ * P, :]
                            .rearrange("(n p) s -> p n s", p=P))
                        xq_slabs.append(xs)
                    if sc == 0:
                        # on the sync queue AFTER chunk-0's x: queue FIFO
                        # keeps these (needed only by the m=4/5 chains /
                        # RoPE, later than chunk-0 x) off the wire until
                        # the first matmul chains' inputs have landed.
                        nc.sync.dma_start(
                            wks[:].rearrange("p (n m) -> p n m", n=NDT),
                            wk_d.rearrange("(n p) m -> p n m", p=P))
                        nc.sync.dma_start(
                            wvs[:].rearrange("p (n m) -> p n m", n=NDT),
                            wv_d.rearrange("(n p) m -> p n m", p=P))
                        if rep == 0:
                            load_consts()

                    def xts_k(k):
                        return xq_slabs[k // XQ][:, (k % XQ) * SC:(k % XQ + 1) * SC]

                    # m = 0..3: q heads; 4: k; 5: v
                    for m in range(NH + 2):
                        psum = ps1.tile([P, SC], FP32, tag="proj", bufs=4)
                        for k in range(NDT):
                            if m < NH:
                                lhsT = wqs[:, k * NH * HD + m * HD:
                                           k * NH * HD + (m + 1) * HD]
                            elif m == NH:
                                lhsT = wks[:, k * HD:(k + 1) * HD]
                            else:
                                lhsT = wvs[:, k * HD:(k + 1) * HD]
                            nc.tensor.matmul(
                                psum[:], lhsT, xts_k(k),
                                start=(k == 0), stop=(k == NDT - 1),
                            )
                        # previous rep's final-chunk output projection:
                        # PE filler for this DMA-bound stretch (copies on
                        # the phase-1-idle scalar/gpsimd engines)
                        if sc < 2:
                            emit_pending(2)
                        if m <= NH:
                            # RoPE: dst = psum*cosT + shift(psum)*sinrotT.
                            # The sin table is partition-PRE-rotated on the
                            # host (sinrot2[d] = sinrot[(d+64)%128]), so
                            # both multiplies are partition-aligned single
                            # DVE ops; the half-swap happens in the final
                            # adds (GpSimd, phase-1 idle), whose crossed
                            # reads undo the rotation.
                            dst = (qt[m] if m < NH else kt)[:, s0:s0 + SC]
                            t0 = rtmp.tile([P, SC], MDT, tag="t0")
                            t1 = rtmp.tile([P, SC], MDT, tag="t1")
                            nc.vector.tensor_mul(
                                t0[:], psum[:], cosT_sb[:, s0:s0 + SC])
                            nc.vector.tensor_mul(
                                t1[0:64, :], psum[64:128, :],
                                sinrT_sb[0:64, s0:s0 + SC])
                            nc.vector.tensor_mul(
                                t1[64:128, :], psum[0:64, :],
                                sinrT_sb[64:128, s0:s0 + SC])
                            nc.gpsimd.tensor_add(dst, t0[:], t1[:])
                        else:
                            nc.vector.tensor_copy(vT[:, s0:s0 + SC], psum[:])

                    # transpose this chunk of V^T -> V tiles [S_k=128, HD]
                    for kk in range(SC // P):
                        k = sc * (SC // P) + kk
                        ps_t = ps1.tile([P, P], MDT, tag="vt")
                        nc.tensor.transpose(
                            ps_t[:], vT[:, k * P:(k + 1) * P], ident_sb[:])
                        nc.vector.tensor_copy(v_tiles[k][:], ps_t[:])

            # ========== phase 2: attention + output projection ==========
            with tc.tile_pool(name="ps2", bufs=2, space="PSUM") as ps2, \
                 tc.tile_pool(name="pss", bufs=2, space="PSUM") as pss:

                # wo re-loaded per rep; AFTER the previous rep's pending
                # oproj (emitted during phase 1 above) has consumed the old
                # contents, and early enough to land before this rep's
                # first oproj groups.
                nc.sync.dma_start(
                    wos[:].rearrange("p (n d) -> p n d", n=NH),
                    wo_d.rearrange("(n p) d -> p n d", p=P))

                def attn_scores(c, h):
                    """scores + exp for (chunk c, head h) -> (pts, offs)."""
                    q0 = c * SC
                    nk = 4 * c + 4
                    pts = []
                    offs = []
                    for k in range(nk):
                        # diagonal blocks: sk tile k only attends to
                        # sq >= 128k, i.e. chunk columns [off:512).
                        # f32r matmuls need moving dim >= 256 for the
                        # 1 cyc/row mode; bf16 has no such constraint.
                        off = max(0, (k - 4 * c) * P)
                        if MDT == F32R:
                            off = min(off, SC - 2 * P)
                        sc_ps = ps2.tile([P, SC], FP32, tag="sc", bufs=3)
                        nc.tensor.matmul(
                            sc_ps[:, off:],
                            kt[:, k * P:(k + 1) * P],
                            qt[h][:, q0 + off:q0 + SC],
                            start=True, stop=True,
                        )
                        pt = ptp.tile([P, SC], MDT, tag="pt")
                        nc.scalar.activation(
                            pt[:, off:], sc_ps[:, off:],
                            mybir.ActivationFunctionType.Exp,
                            scale=SCALE)
                        if k >= 4 * c:
                            # diagonal block: only the first 128-col
                            # sub-block is triangular, and its masked-out
                            # entries are real (bounded) scores, so exp
                            # runs over the whole tile and a cheap 0/1
                            # mask multiply (2x-mode DVE) zeroes them.
                            r = k % NCH
                            m1 = min(off + P, SC)
                            nc.vector.tensor_mul(
                                pt[:, off:m1], pt[:, off:m1],
                                mdiag_sb[r][:, off:m1])
                        pts.append(pt)
                        offs.append(off)
                    return pts, offs

                def attn_tail(c, h, pts, offs):
                    """sums + AV + normalize for (chunk c, head h)."""
                    nk = 4 * c + 4
                    # all-ones stationary -> every psum partition gets
                    # the column sum over sk (broadcast for free)
                    sums_ps = pss.tile([P, SC], FP32, tag="sums", bufs=2)
                    # softmax denominators: off-diagonal prob tiles are
                    # full-width, so pre-accumulate them in chains on the
                    # otherwise-idle GpSimd / DVE engines; the PE then does
                    # one ones-matmul per chain instead of one per tile.
                    # Diagonal tiles (partial width) stay as individual
                    # accumulating ones-matmuls.  GpSimd (~3x slower per
                    # add) gets one add on the earliest-produced tiles; DVE
                    # takes the rest.
                    noff = 4 * c
                    sum_items = []
                    if noff >= 2:
                        split = min(2, noff - 2) if noff > 2 else noff
                        for ci, (eng, ks) in enumerate((
                                (nc.gpsimd, list(range(0, split))),
                                (nc.vector, list(range(split, noff))))):
                            if not ks:
                                continue
                            if len(ks) == 1:
                                sum_items.append((pts[ks[0]], 0))
                                continue
                            acc = chp.tile([P, SC], MDT, tag=f"ch{ci}")
                            eng.tensor_add(
                                acc[:], pts[ks[0]][:], pts[ks[1]][:])
                            for k in ks[2:]:
                                eng.tensor_add(acc[:], acc[:], pts[k][:])
                            sum_items.append((acc, 0))
                    sum_items += [(pts[k], offs[k]) for k in range(noff, nk)]
                    for i, (t, off) in enumerate(sum_items):
                        nc.tensor.matmul(
                            sums_ps[:, off:], ones_sb[:], t[:, off:],
                            start=(i == 0), stop=(i == len(sum_items) - 1),
                        )
                    # AV: outT_h [HD, sq] = sum_k V_k^T @ probsT_k
                    av_ps = ps2.tile([P, SC], FP32, tag="av", bufs=1)
                    for k in range(nk):
                        nc.tensor.matmul(
                            av_ps[:, offs[k]:], v_tiles[k][:],
                            pts[k][:, offs[k]:],
                            start=(k == 0), stop=(k == nk - 1),
                        )
                    # normalize: outT = av * (1/sums); the mul is split in
                    # halves so the output projection (which reads outT in
                    # 128-col t-slices) can start after the first half.
                    ot = p2.tile([P, SC], MDT, tag=f"ot{h}", bufs=2)
                    recip = nrm.tile([P, SC], FP32, tag="recip")
                    nc.vector.reciprocal(recip[:], sums_ps[:])
                    hw = SC // 2
                    nc.vector.tensor_mul(
                        ot[:, 0:hw], av_ps[:, 0:hw], recip[:, 0:hw])
                    nc.vector.tensor_mul(
                        ot[:, hw:], av_ps[:, hw:], recip[:, hw:])
                    return ot

                # chunk c's attention is ScalarE(exp)-throughput-bound at
                # large c, so the previous chunk's output-projection
                # matmuls are interleaved into its instruction stream as
                # PE filler, between the scores and tails of each head
                # group, where the PE would otherwise wait on exp results.
                for c in range(NCH):
                    nk = 4 * c + 4
                    # heads interleave: later heads' scores give the PE
                    # independent work while earlier heads' softmax tails
                    # complete.  Group size bounded by the prob-tile pool.
                    grp = max(1, min(NH, 32 // nk))
                    points = 2 * (NH // grp)
                    per = -(-len(oproj_items) // points)
                    otiles = {}
                    for hp in range(0, NH, grp):
                        done = []
                        for h in range(hp, min(hp + grp, NH)):
                            done.append((h, *attn_scores(c, h)))
                        emit_pending(per)
                        for h, pts, offs in done:
                            otiles[h] = attn_tail(c, h, pts, offs)
                        emit_pending(per)
                    # anything left of the previous chunk, then hand off
                    emit_pending(len(oproj_items))
                    pend["cur"] = {"c": c, "ot": otiles, "idx": 0}

        # drain the last rep's final-chunk output projection
        emit_pending(len(oproj_items))

    nc.compile()
    return nc


def _host_tables():
    inv_freq = 1.0 / (THETA ** (np.arange(0, HD, 2, dtype=np.float32) / HD))
    t = np.arange(S, dtype=np.float32)
    freqs = t[:, None] * inv_freq[None, :]              # [S, HD/2]
    emb = np.concatenate([freqs, freqs], axis=-1)       # [S, HD]
    cos = np.cos(emb).astype(np.float32)
    sin = np.sin(emb).astype(np.float32)
    cosT = np.ascontiguousarray(cos.T)                  # [HD, S]
    sinT = np.ascontiguousarray(sin.T)
    sinrotT = sinT.copy()
    sinrotT[0:HD // 2] = -sinT[0:HD // 2]
    return cosT, sinrotT


def get_program(mm_mode=MM_MODE, repeat=1):
    key = ("nc", mm_mode, repeat)
    if key not in _CACHE:
        _CACHE[key] = _build_program(mm_mode, repeat)
    return _CACHE[key]


def make_in_maps(x, wq, wk, wv, wo, mask, mm_mode=MM_MODE):
    if mm_mode == "bf16":
        import ml_dtypes
        mdt_np = np.dtype(ml_dtypes.bfloat16)
    else:
        mdt_np = np.dtype(np.float32)
    x = np.asarray(x, dtype=np.float32)
    wq = np.asarray(wq, dtype=mdt_np)
    wk = np.asarray(wk, dtype=mdt_np)
    wv = np.asarray(wv, dtype=mdt_np)
    wo = np.asarray(wo, dtype=mdt_np)
    mask = np.asarray(mask, dtype=np.float32)

    cosT, sinrotT = _host_tables()
    ident = np.eye(P, dtype=np.float32)
    # maskdiag[r*128+a, b] = 1 where mask[0,0, b, r*128+a] == 0 (visible),
    # else 0; multiplied in post-exp.  Pattern repeats per chunk.
    maskdiag = np.ascontiguousarray(
        (mask[0, 0, 0:SC, 0:SC].T == 0.0)).astype(mdt_np)

    # chunk-contiguous xT: block sc holds xT[:, sc*SC:(sc+1)*SC] as rows
    # [sc*D:(sc+1)*D], so device DMA rows are contiguous in DRAM
    xTc = [np.ascontiguousarray(
               x[b].T.astype(mdt_np).reshape(D, NCH, SC)
               .transpose(1, 0, 2)).reshape(NCH * D, SC)
           for b in range(B)]
    in_maps = []
    for c in range(NCORES):
        b, g = c // NG, c % NG
        qc0 = g * NH * HD
        kc0 = g * HD
        in_maps.append({
            "xTc": xTc[b],
            "wqg": np.ascontiguousarray(wq[:, qc0:qc0 + NH * HD]),
            "wkg": np.ascontiguousarray(wk[:, kc0:kc0 + HD]),
            "wvg": np.ascontiguousarray(wv[:, kc0:kc0 + HD]),
            "wog": np.ascontiguousarray(wo[qc0:qc0 + NH * HD, :]),
            "cosT": cosT.astype(mdt_np),
            "sinrotT": sinrotT.astype(mdt_np),
            "maskdiag": maskdiag,
            "ident": ident.astype(mdt_np),
            "ones": np.ones((P, P), dtype=mdt_np),
        })
    return in_maps


LAST_RESULTS = None


def _make_exec(nc):
    """Mirror run_bass_via_pjrt's multi-core path, but keep the jitted
    executable so repeated (timed) dispatches skip retrace/reload."""
    import jax
    from jax.experimental.shard_map import shard_map
    from jax.sharding import Mesh, PartitionSpec

    from concourse import bass2jax, mybir as _mybir

    bass2jax.install_neuronx_cc_hook()
    partition_name = (
        nc.partition_id_tensor.name if nc.partition_id_tensor else None)
    in_names, out_names, out_avals, zero_outs = [], [], [], []
    for alloc in nc.m.functions[0].allocations:
        if not isinstance(alloc, _mybir.MemoryLocationSet):
            continue
        name = alloc.memorylocations[0].name
        if alloc.kind == "ExternalInput":
            if name != partition_name:
                in_names.append(name)
        elif alloc.kind == "ExternalOutput":
            shape = tuple(alloc.tensor_shape)
            dtype = _mybir.dt.np(alloc.dtype)
            out_names.append(name)
            out_avals.append(jax.core.ShapedArray(shape, dtype))
            zero_outs.append(np.zeros(shape, dtype))
    n_params = len(in_names)
    n_outs = len(out_avals)
    all_in_names = list(in_names) + list(out_names)
    if partition_name is not None:
        all_in_names.append(partition_name)
    donate = tuple(range(n_params, n_params + n_outs))

    def _body(*args):
        operands = list(args)
        if partition_name is not None:
            operands.append(bass2jax.partition_id_tensor())
        outs = bass2jax._bass_exec_p.bind(
            *operands,
            out_avals=tuple(out_avals),
            in_names=tuple(all_in_names),
            out_names=tuple(out_names),
            lowering_input_output_aliases=(),
            sim_require_finite=True,
            sim_require_nnan=True,
            nc=nc,
        )
        return tuple(outs)

    devices = jax.devices()[:NCORES]
    mesh = Mesh(np.asarray(devices), ("core",))
    sharded = jax.jit(
        shard_map(
            _body, mesh=mesh,
            in_specs=(PartitionSpec("core"),) * (n_params + n_outs),
            out_specs=(PartitionSpec("core"),) * n_outs,
            check_rep=False,
        ),
        donate_argnums=donate, keep_unused=True,
    )
    return {
        "fn": sharded, "in_names": in_names, "out_names": out_names,
        "out_avals": out_avals, "zero_outs": zero_outs, "mesh": mesh,
    }


def get_exec(mm_mode=MM_MODE, repeat=1):
    key = ("exec", mm_mode, repeat)
    if key not in _CACHE:
        _CACHE[key] = _make_exec(get_program(mm_mode, repeat))
    return _CACHE[key]


def _concat_inputs(ex, in_maps):
    return [
        np.concatenate([np.asarray(in_maps[c][name]) for c in range(NCORES)],
                       axis=0)
        for name in ex["in_names"]
    ]


def _concat_zeros(ex):
    return [
        np.zeros((NCORES * z.shape[0], *z.shape[1:]), z.dtype)
        for z in ex["zero_outs"]
    ]


def run_on_device(in_maps, mm_mode=MM_MODE, repeat=1):
    """One dispatch; returns per-core output dicts (numpy)."""
    ex = get_exec(mm_mode, repeat)
    out_arrs = ex["fn"](*_concat_inputs(ex, in_maps), *_concat_zeros(ex))
    res = []
    for c in range(NCORES):
        res.append({
            name: np.asarray(out_arrs[i]).reshape(
                NCORES, *ex["out_avals"][i].shape)[c]
            for i, name in enumerate(ex["out_names"])
        })
    return res


def bench(in_maps, iters=5, mm_mode=MM_MODE, repeat=1):
    """Timed repeated dispatch: inputs pre-placed on device, fresh donated
    zero output buffers pre-placed per iteration. Returns list of wall ns."""
    import time

    import jax
    from jax.sharding import NamedSharding, PartitionSpec

    ex = get_exec(mm_mode, repeat)
    sh = NamedSharding(ex["mesh"], PartitionSpec("core"))
    dev_in = [jax.device_put(a, sh) for a in _concat_inputs(ex, in_maps)]
    zsets = [[jax.device_put(z, sh) for z in _concat_zeros(ex)]
             for _ in range(iters + 1)]
    jax.block_until_ready(dev_in)
    jax.block_until_ready(zsets)
    out = ex["fn"](*dev_in, *zsets[0])       # warm-up
    jax.block_until_ready(out)
    times = []
    for i in range(iters):
        t0 = time.perf_counter()
        out = ex["fn"](*dev_in, *zsets[i + 1])
        jax.block_until_ready(out)
        times.append((time.perf_counter() - t0) * 1e9)
    return times


def bench_slope(in_maps, iters=8, mm_mode=MM_MODE, r_hi=4):
    """Per-iteration kernel time via slope: (T(r_hi) - T(1)) / (r_hi - 1).
    Immune to constant dispatch overhead."""
    t1 = bench(in_maps, iters=iters, mm_mode=mm_mode, repeat=1)
    th = bench(in_maps, iters=iters, mm_mode=mm_mode, repeat=r_hi)
    t1m, thm = np.median(t1), np.median(th)
    t1b, thb = min(t1), min(th)
    return {
        "t1": t1, "th": th,
        "exec_ns_median": (thm - t1m) / (r_hi - 1),
        "exec_ns_min": (thb - t1b) / (r_hi - 1),
    }


def kernel(x, wq, wk, wv, wo, mask):
    """Full inputs in, full output out; shards over the 8 NeuronCores."""
    global LAST_RESULTS
    from concourse import bass_utils

    nc = get_program()
    in_maps = make_in_maps(x, wq, wk, wv, wo, mask)
    res = bass_utils.run_bass_kernel_spmd(
        nc, in_maps, core_ids=list(range(NCORES)))
    LAST_RESULTS = res
    out = np.zeros((B, S, D), dtype=np.float32)
    for c in range(NCORES):
        b = c // NG
        yb = np.asarray(res.results[c]["ybc"]).reshape(NCH, S, SC)
        for dci in range(NCH):
            out[b, :, dci * SC:(dci + 1) * SC] += yb[dci].astype(np.float32)
    return out
